# revision 1
# baseline (speedup 1.0000x reference)
"""Trainium2 Bass kernel for nn_Attention_11046655885816.

Full inputs in, full output out. Internally: 8 NeuronCores, each core
handles (one batch, a slice of heads). Projections + attention run
on-device in fp16/bf16 with fp32 PSUM accumulation; the softmax
denominator is produced by appending a key-mask column to the value
matrix, and the final divide + head assembly happens on the host.

Key layout choices (per core):
  qT, kT   : [64*NH partitions (head-major), L]  (fp16)  -> scores need no
             transposes anywhere: S^T tile = kT_tile.T @ qT.
  v_aug    : [Lk partitions, NH*(64+1)]  (bf16) -- per head 64 value cols
             plus one kmask column; AV matmul then yields numerator and
             denominator in one accumulation group.
  exp      : ScalarE reads score PSUM quads [128, 3*512] directly and
             writes bf16 T tiles to SBUF.
No max-subtraction is needed: scores are O(+-60) and exp stays inside
fp32/bf16 range; masked keys contribute exactly zero via the zeroed
v_aug rows (V_seq columns are zeroed host-side past V_len).
"""

import math
import os
import numpy as np
import ml_dtypes

B, L, D = 4, 2048, 1024
H, DH = 16, 64

_nc_cache = {}
LAST_EXEC_NS = None
LAST_SPMD_WALL_NS = None
LAST_RESULT = None


def _build(cfg):
    """Build + compile the per-core Bass program for a launch config.

    cfg keys: NH (heads/core, even), LQ, LK (multiples of 128).
    """
    import concourse.bass as bass
    import concourse.mybir as mybir
    import concourse.tile as tile
    from concourse import bacc

    NH = cfg["NH"]
    LQ = cfg["LQ"]
    LK = cfg["LK"]
    assert NH % 2 == 0 and LQ % 128 == 0 and LK % 128 == 0
    EH = NH * DH                 # E columns on this core
    NEB = EH // 128              # E blocks == head pairs
    ND = D // 128                # contraction tiles for projections
    NTK = LK // 128              # lk tiles
    NLQB = LQ // 128             # lq blocks
    VW = DH + 1                  # value cols + mask col per head

    # lk quads: up to 8 tiles of [128, 128] packed into one [128, 1024]
    # 2-bank PSUM region (scores for one 128-wide lq block); 2-bank quads
    # leave room for a dedicated projection PSUM pool so k/q projection
    # overlaps attention instead of fighting for the score slots
    quads = []
    t = 0
    while t < NTK:
        n = min(4, NTK - t)
        quads.append((t, n))
        t += n

    fp16 = mybir.dt.float16
    bf16 = mybir.dt.bfloat16
    f32 = mybir.dt.float32

    # Per-head-pair arena strides padded to 8 KiB: base_partition=64
    # matmul operands at free-offsets that are odd multiples of 4 KiB
    # returned corrupted scores on HW; 8 KiB-aligned slices are clean.
    LKS = ((LK * 2 + 8191) // 8192) * 4096
    LQS = ((LQ * 2 + 8191) // 8192) * 4096

    nc = bacc.Bacc(
        "TRN2", target_bir_lowering=False, debug=False, num_devices=8
    )

    xq = nc.dram_tensor("xq", [D, LQ], fp16, kind="ExternalInput").ap()
    xk = nc.dram_tensor("xk", [D, LK], fp16, kind="ExternalInput").ap()
    xv = nc.dram_tensor("xv", [D, LK], fp16, kind="ExternalInput").ap()
    wq = nc.dram_tensor("wq", [D, EH], fp16, kind="ExternalInput").ap()
    wk = nc.dram_tensor("wk", [D, EH], fp16, kind="ExternalInput").ap()
    wv = nc.dram_tensor("wv", [D, EH], fp16, kind="ExternalInput").ap()
    km = nc.dram_tensor("km", [128, NTK * NH], bf16, kind="ExternalInput").ap()
    outp = nc.dram_tensor("outp", [LQ, NH * VW], f32, kind="ExternalOutput").ap()

    with tile.TileContext(nc, trace_sim=False) as tc:
        with (
            tc.tile_pool(name="xc", bufs=3) as xc_pool,
            tc.tile_pool(name="win", bufs=1) as win_pool,
            tc.tile_pool(name="proj", bufs=1) as proj_pool,
            tc.tile_pool(name="tsb", bufs=6) as t_pool,
            tc.tile_pool(name="osb", bufs=8) as o_pool,
            tc.tile_pool(name="ps", bufs=2, space="PSUM") as pp_pool,
            tc.tile_pool(name="pav", bufs=2, space="PSUM") as pav_pool,
            tc.tile_pool(name="pj", bufs=2, space="PSUM") as pj_pool,
        ):
            # ---- persistent SBUF arenas ----
            wq_sb = win_pool.tile([128, ND * EH], fp16, tag="wq")
            wk_sb = win_pool.tile([128, ND * EH], fp16, tag="wk")
            wv_sb = win_pool.tile([128, ND * EH], fp16, tag="wv")
            qt_sb = proj_pool.tile([128, NEB * LQS], fp16, tag="qt")
            kt_sb = proj_pool.tile([128, NEB * LKS], fp16, tag="kt")
            v_sb = proj_pool.tile([128, NTK * NH * VW], bf16, tag="v")

            # ---- weight + kmask DMAs ----
            for dt in range(ND):
                nc.sync.dma_start(
                    wv_sb[:, dt * EH : (dt + 1) * EH],
                    wv[dt * 128 : (dt + 1) * 128, :],
                )
                nc.sync.dma_start(
                    wk_sb[:, dt * EH : (dt + 1) * EH],
                    wk[dt * 128 : (dt + 1) * 128, :],
                )
                nc.sync.dma_start(
                    wq_sb[:, dt * EH : (dt + 1) * EH],
                    wq[dt * 128 : (dt + 1) * 128, :],
                )
            v4 = v_sb[:].rearrange("p (t h c) -> p t h c", t=NTK, h=NH, c=VW)
            nc.sync.dma_start(
                v4[:, :, :, DH],
                km.rearrange("p (t h) -> p t h", h=NH),
            )

            def stream_x(src):
                """DMA one 512-wide L-chunk of all D-tiles into a fresh tile."""
                def get(lc, w):
                    xc = xc_pool.tile([128, ND * 512], fp16, tag="xc")
                    for dt in range(ND):
                        nc.sync.dma_start(
                            xc[:, dt * 512 : dt * 512 + w],
                            src[dt * 128 : (dt + 1) * 128, lc : lc + w],
                        )
                    return xc
                return get

            get_xv = stream_x(xv)
            get_xk = stream_x(xk)
            get_xq = stream_x(xq)

            # ---- projections ----
            def proj_v():
                # v: normal layout [lk, E]; stationary = xv tile, moving = wv
                for lc in range(0, LK, 512):
                    w = min(512, LK - lc)
                    xcv = get_xv(lc, w)
                    for t4 in range((w + 127) // 128):
                        t = lc // 128 + t4
                        ps = pj_pool.tile([128, 512], f32, tag="pj")
                        for dt in range(ND):
                            nc.tensor.matmul(
                                ps[:, :EH],
                                lhsT=xcv[:, dt * 512 + t4 * 128 : dt * 512 + (t4 + 1) * 128],
                                rhs=wv_sb[:, dt * EH : (dt + 1) * EH],
                                start=(dt == 0),
                                stop=(dt == ND - 1),
                            )
                        nc.vector.tensor_copy(
                            v4[:, t, :, 0:DH],
                            ps[:, :EH].rearrange("p (h e) -> p h e", h=NH, e=DH),
                        )

            def proj_kq(eb):
                # k, q: transposed layout [E, L]; stationary = W block
                for lc in range(0, LK, 512):
                    w = min(512, LK - lc)
                    xck = get_xk(lc, w)
                    ps = pj_pool.tile([128, 512], f32, tag="pj")
                    for dt in range(ND):
                        nc.tensor.matmul(
                            ps[:, :w],
                            lhsT=wk_sb[:, dt * EH + eb * 128 : dt * EH + (eb + 1) * 128],
                            rhs=xck[:, dt * 512 : dt * 512 + w],
                            start=(dt == 0),
                            stop=(dt == ND - 1),
                        )
                    nc.vector.tensor_copy(
                        kt_sb[:, eb * LKS + lc : eb * LKS + lc + w], ps[:, :w]
                    )
                for lc in range(0, LQ, 512):
                    w = min(512, LQ - lc)
                    xcq = get_xq(lc, w)
                    ps = pj_pool.tile([128, 512], f32, tag="pj")
                    for dt in range(ND):
                        nc.tensor.matmul(
                            ps[:, :w],
                            lhsT=wq_sb[:, dt * EH + eb * 128 : dt * EH + (eb + 1) * 128],
                            rhs=xcq[:, dt * 512 : dt * 512 + w],
                            start=(dt == 0),
                            stop=(dt == ND - 1),
                        )
                    nc.vector.tensor_copy(
                        qt_sb[:, eb * LQS + lc : eb * LQS + lc + w], ps[:, :w]
                    )

            # ---- attention, with projection of the NEXT head pair
            # interleaved so it hides under this pair's ScalarE exps ----
            # lq handled in PAIRS of 128-blocks: scores at N=256 halve the
            # PE matmul/LDW count; T persists per pair-iteration and the
            # two AV passes share the 2 accumulator banks sequentially.
            proj_kq(0)
            proj_v()
            for hp in range(NEB):
                hA, hB = 2 * hp, 2 * hp + 1
                for lqs in range(0, LQ, 256):
                    w = min(256, LQ - lqs)
                    nlqb = w // 128
                    tA = t_pool.tile([128, NTK * 256], bf16, tag="t")
                    tB = t_pool.tile([128, NTK * 256], bf16, tag="t")
                    for (t0, tn) in quads:
                        psA = pp_pool.tile([128, 1024], f32, tag="sq")
                        psB = pp_pool.tile([128, 1024], f32, tag="sq")
                        for j in range(tn):
                            tt = t0 + j
                            nc.tensor.matmul(
                                psA[:, j * w : (j + 1) * w],
                                lhsT=kt_sb[0:64, hp * LKS + tt * 128 : hp * LKS + (tt + 1) * 128],
                                rhs=qt_sb[0:64, hp * LQS + lqs : hp * LQS + lqs + w],
                                start=True,
                                stop=True,
                            )
                            nc.tensor.matmul(
                                psB[:, j * w : (j + 1) * w],
                                lhsT=kt_sb[64:128, hp * LKS + tt * 128 : hp * LKS + (tt + 1) * 128],
                                rhs=qt_sb[64:128, hp * LQS + lqs : hp * LQS + lqs + w],
                                start=True,
                                stop=True,
                            )
                        w_all = tn * w
                        nc.scalar.activation(
                            tA[:, t0 * w : t0 * w + w_all], psA[:, :w_all],
                            mybir.ActivationFunctionType.Exp,
                        )
                        nc.scalar.activation(
                            tB[:, t0 * w : t0 * w + w_all], psB[:, :w_all],
                            mybir.ActivationFunctionType.Exp,
                        )
                    for lb in range(nlqb):
                        pavA = pav_pool.tile([128, VW], f32, tag="av")
                        pavB = pav_pool.tile([128, VW], f32, tag="av")
                        for tt in range(NTK):
                            nc.tensor.matmul(
                                pavA[:, 0:VW],
                                lhsT=tA[:, tt * w + lb * 128 : tt * w + lb * 128 + 128],
                                rhs=v4[:, tt, hA, :],
                                start=(tt == 0),
                                stop=(tt == NTK - 1),
                            )
                            nc.tensor.matmul(
                                pavB[:, 0:VW],
                                lhsT=tB[:, tt * w + lb * 128 : tt * w + lb * 128 + 128],
                                rhs=v4[:, tt, hB, :],
                                start=(tt == 0),
                                stop=(tt == NTK - 1),
                            )
                        oA = o_pool.tile([128, VW], f32, tag="o")
                        oB = o_pool.tile([128, VW], f32, tag="o")
                        nc.vector.tensor_copy(oA[:, :], pavA[:, :])
                        nc.vector.tensor_copy(oB[:, :], pavB[:, :])
                        ls = lqs + lb * 128
                        nc.sync.dma_start(
                            outp[ls : ls + 128, hA * VW : (hA + 1) * VW], oA[:, :]
                        )
                        nc.sync.dma_start(
                            outp[ls : ls + 128, hB * VW : (hB + 1) * VW], oB[:, :]
                        )
                if hp + 1 < NEB:
                    proj_kq(hp + 1)

    nc.compile()
    return nc


def _get_nc(cfg):
    key = tuple(sorted(cfg.items()))
    if key not in _nc_cache:
        _nc_cache[key] = _build(cfg)
    return _nc_cache[key]


def _prep_core_inputs(Xq, Xk, Xv, Wq, Wk, Wv, vlen, cfg):
    """Host-side slicing/transposition/masking for one core.

    Xq/Xk/Xv: [L, D] fp32 for this batch; W*: [D, EH] slices for this
    core's heads; vlen: effective V_len (0 means "no mask").
    """
    NH, LQ, LK = cfg["NH"], cfg["LQ"], cfg["LK"]
    f16 = np.float16
    bf16 = ml_dtypes.bfloat16

    NTK = LK // 128
    xq = np.zeros((D, LQ), f16)
    xq[:, : min(LQ, L)] = Xq[: min(LQ, L)].T.astype(f16)
    xk = np.zeros((D, LK), f16)
    xv = np.zeros((D, LK), f16)
    n = min(LK, L) if vlen == 0 else min(LK, vlen)
    xk[:, :n] = Xk[:n].T.astype(f16)
    xv[:, :n] = Xv[:n].T.astype(f16)
    kmask = (np.arange(LK) < n).astype(np.float32)
    # device layout [128, NTK*NH]: km[p, t*NH + h] = kmask[t*128 + p]
    kmv = np.repeat(
        kmask.reshape(NTK, 128).T[:, :, None], NH, axis=2
    ).reshape(128, NTK * NH)
    return {
        "xq": xq,
        "xk": xk,
        "xv": xv,
        "wq": np.ascontiguousarray(Wq, dtype=f16),
        "wk": np.ascontiguousarray(Wk, dtype=f16),
        "wv": np.ascontiguousarray(Wv, dtype=f16),
        "km": kmv.astype(bf16),
    }


def kernel(Q_seq, K_seq, V_seq, Q_len, V_len, WQ, WK, WV):
    from concourse.bass_utils import run_bass_kernel_spmd

    Q_seq = np.asarray(Q_seq, np.float32)
    K_seq = np.asarray(K_seq, np.float32)
    V_seq = np.asarray(V_seq, np.float32)
    WQ = np.asarray(WQ, np.float32)
    WK = np.asarray(WK, np.float32)
    WV = np.asarray(WV, np.float32)
    q_len = np.asarray(Q_len).reshape(-1).astype(np.int64)
    v_len = np.asarray(V_len).reshape(-1).astype(np.int64)

    # LQ covers the largest Q_len (batch 2: 1748); rows beyond each
    # batch's Q_len are dropped host-side anyway. LK must cover V_len.
    lq_need = int(min(L, max(1, q_len.max())))
    lk_need = int(min(L, max(v_len.max(), 1)))
    if (v_len == 0).any():
        lk_need = L
    cfg = {
        "NH": 8,
        "LQ": ((lq_need + 127) // 128) * 128,
        "LK": ((lk_need + 127) // 128) * 128,
    }
    NH, LQ, LK = cfg["NH"], cfg["LQ"], cfg["LK"]
    VW = DH + 1
    nc = _get_nc(cfg)

    in_maps = []
    core_meta = []
    for b in range(B):
        for hg in range(2):
            e0, e1 = hg * NH * DH, (hg + 1) * NH * DH
            m = _prep_core_inputs(
                Q_seq[b], K_seq[b], V_seq[b],
                WQ[:, e0:e1], WK[:, e0:e1], WV[:, e0:e1],
                int(v_len[b]), cfg,
            )
            in_maps.append(m)
            core_meta.append((b, hg))

    import time as _time

    trace = os.environ.get("NN_ATT_TRACE") == "1"
    t_spmd = _time.time()
    try:
        res = run_bass_kernel_spmd(
            nc, in_maps, core_ids=list(range(8)), trace=trace,
            **({"trace_cores": list(range(8))} if trace else {}),
        )
    except Exception:
        if not trace:
            raise
        res = run_bass_kernel_spmd(nc, in_maps, core_ids=list(range(8)))
    global LAST_EXEC_NS, LAST_RESULT, LAST_SPMD_WALL_NS
    LAST_SPMD_WALL_NS = int((_time.time() - t_spmd) * 1e9)
    LAST_RESULT = res
    if res.exec_time_ns:
        LAST_EXEC_NS = int(res.exec_time_ns)

    out = np.zeros((B, L, H * DH), np.float32)
    for c, (b, hg) in enumerate(core_meta):
        arr = res.results[c]["outp"]  # [LQ, NH*VW]
        nq = min(int(q_len[b]), LQ, L)
        if nq <= 0:
            continue
        a = arr[:nq].reshape(nq, NH, VW)
        num = a[:, :, :DH]
        den = a[:, :, DH:DH + 1]
        o = num / den
        out[b, :nq, hg * NH * DH : (hg + 1) * NH * DH] = o.reshape(nq, NH * DH)
    return out



# revision 7
# speedup vs baseline: 5.8664x; 5.8664x over previous
"""Trainium2 Bass kernel for nn_Attention_11046655885816.

Full inputs in, full output out. Internally: 8 NeuronCores, each core
handles (one batch, a slice of heads). Projections + attention run
on-device in fp16/bf16 with fp32 PSUM accumulation; the softmax
denominator is produced by appending a key-mask column to the value
matrix, and the final divide + head assembly happens on the host.

Key layout choices (per core):
  qT, kT   : [64*NH partitions (head-major), L]  (fp16)  -> scores need no
             transposes anywhere: S^T tile = kT_tile.T @ qT.
  v_aug    : [Lk partitions, NH*(64+1)]  (bf16) -- per head 64 value cols
             plus one kmask column; AV matmul then yields numerator and
             denominator in one accumulation group.
  exp      : ScalarE reads score PSUM quads [128, 3*512] directly and
             writes bf16 T tiles to SBUF.
No max-subtraction is needed: scores are O(+-60) and exp stays inside
fp32/bf16 range; masked keys contribute exactly zero via the zeroed
v_aug rows (V_seq columns are zeroed host-side past V_len).
"""

import math
import os
import numpy as np
import ml_dtypes

B, L, D = 4, 2048, 1024
H, DH = 16, 64

_nc_cache = {}
LAST_EXEC_NS = None
LAST_SPMD_WALL_NS = None
LAST_RESULT = None


def _build(cfg):
    """Build + compile the per-core Bass program for a launch config.

    cfg keys: NH (heads/core, even), LQ, LK (multiples of 128).
    """
    import concourse.bass as bass
    import concourse.mybir as mybir
    import concourse.tile as tile
    from concourse import bacc

    NH = cfg["NH"]
    LQ = cfg["LQ"]
    LK = cfg["LK"]
    assert NH % 2 == 0 and LQ % 128 == 0 and LK % 128 == 0
    EH = NH * DH                 # E columns on this core
    NEB = EH // 128              # E blocks == head pairs
    ND = D // 128                # contraction tiles for projections
    NTK = LK // 128              # lk tiles
    NLQB = LQ // 128             # lq blocks
    VW = DH + 1                  # value cols + mask col per head

    # lk quads: up to 8 tiles of [128, 128] packed into one [128, 1024]
    # 2-bank PSUM region (scores for one 128-wide lq block); 2-bank quads
    # leave room for a dedicated projection PSUM pool so k/q projection
    # overlaps attention instead of fighting for the score slots
    quads = []
    t = 0
    while t < NTK:
        n = min(4, NTK - t)
        quads.append((t, n))
        t += n

    fp16 = mybir.dt.float16
    bf16 = mybir.dt.bfloat16
    f32 = mybir.dt.float32

    # Per-head-pair arena strides padded to 8 KiB: base_partition=64
    # matmul operands at free-offsets that are odd multiples of 4 KiB
    # returned corrupted scores on HW; 8 KiB-aligned slices are clean.
    LKS = ((LK * 2 + 8191) // 8192) * 4096
    LQS = ((LQ * 2 + 8191) // 8192) * 4096

    nc = bacc.Bacc(
        "TRN2", target_bir_lowering=False, debug=False, num_devices=8
    )

    xq = nc.dram_tensor("xq", [D, LQ], fp16, kind="ExternalInput").ap()
    xk = nc.dram_tensor("xk", [D, LK], fp16, kind="ExternalInput").ap()
    xv = nc.dram_tensor("xv", [D, LK], fp16, kind="ExternalInput").ap()
    wq = nc.dram_tensor("wq", [D, EH], fp16, kind="ExternalInput").ap()
    wk = nc.dram_tensor("wk", [D, EH], fp16, kind="ExternalInput").ap()
    wv = nc.dram_tensor("wv", [D, EH], fp16, kind="ExternalInput").ap()
    km = nc.dram_tensor("km", [128, NTK * NH], bf16, kind="ExternalInput").ap()
    outp = nc.dram_tensor("outp", [LQ, NH * VW], f32, kind="ExternalOutput").ap()

    with tile.TileContext(nc, trace_sim=False) as tc:
        with (
            tc.tile_pool(name="xc", bufs=3) as xc_pool,
            tc.tile_pool(name="win", bufs=1) as win_pool,
            tc.tile_pool(name="proj", bufs=1) as proj_pool,
            tc.tile_pool(name="tsb", bufs=6) as t_pool,
            tc.tile_pool(name="osb", bufs=8) as o_pool,
            tc.tile_pool(name="ps", bufs=2, space="PSUM") as pp_pool,
            tc.tile_pool(name="pav", bufs=2, space="PSUM") as pav_pool,
            tc.tile_pool(name="pj", bufs=2, space="PSUM") as pj_pool,
        ):
            # ---- persistent SBUF arenas ----
            wq_sb = win_pool.tile([128, ND * EH], fp16, tag="wq")
            wk_sb = win_pool.tile([128, ND * EH], fp16, tag="wk")
            wv_sb = win_pool.tile([128, ND * EH], fp16, tag="wv")
            qt_sb = proj_pool.tile([128, NEB * LQS], fp16, tag="qt")
            kt_sb = proj_pool.tile([128, NEB * LKS], fp16, tag="kt")
            v_sb = proj_pool.tile([128, NTK * NH * VW], bf16, tag="v")

            # ---- weight + kmask DMAs ----
            for dt in range(ND):
                nc.sync.dma_start(
                    wv_sb[:, dt * EH : (dt + 1) * EH],
                    wv[dt * 128 : (dt + 1) * 128, :],
                )
                nc.sync.dma_start(
                    wk_sb[:, dt * EH : (dt + 1) * EH],
                    wk[dt * 128 : (dt + 1) * 128, :],
                )
                nc.sync.dma_start(
                    wq_sb[:, dt * EH : (dt + 1) * EH],
                    wq[dt * 128 : (dt + 1) * 128, :],
                )
            v4 = v_sb[:].rearrange("p (t h c) -> p t h c", t=NTK, h=NH, c=VW)
            nc.sync.dma_start(
                v4[:, :, :, DH],
                km.rearrange("p (t h) -> p t h", h=NH),
            )

            def stream_x(src):
                """DMA one 512-wide L-chunk of all D-tiles into a fresh tile."""
                def get(lc, w):
                    xc = xc_pool.tile([128, ND * 512], fp16, tag="xc")
                    for dt in range(ND):
                        nc.sync.dma_start(
                            xc[:, dt * 512 : dt * 512 + w],
                            src[dt * 128 : (dt + 1) * 128, lc : lc + w],
                        )
                    return xc
                return get

            get_xv = stream_x(xv)
            get_xk = stream_x(xk)
            get_xq = stream_x(xq)

            # ---- projections ----
            def proj_v():
                # v: normal layout [lk, E]; stationary = xv tile, moving = wv
                for lc in range(0, LK, 512):
                    w = min(512, LK - lc)
                    xcv = get_xv(lc, w)
                    for t4 in range((w + 127) // 128):
                        t = lc // 128 + t4
                        ps = pj_pool.tile([128, 512], f32, tag="pj")
                        for dt in range(ND):
                            nc.tensor.matmul(
                                ps[:, :EH],
                                lhsT=xcv[:, dt * 512 + t4 * 128 : dt * 512 + (t4 + 1) * 128],
                                rhs=wv_sb[:, dt * EH : (dt + 1) * EH],
                                start=(dt == 0),
                                stop=(dt == ND - 1),
                            )
                        nc.vector.tensor_copy(
                            v4[:, t, :, 0:DH],
                            ps[:, :EH].rearrange("p (h e) -> p h e", h=NH, e=DH),
                        )

            def proj_kq(eb):
                # k, q: transposed layout [E, L]; stationary = W block
                for lc in range(0, LK, 512):
                    w = min(512, LK - lc)
                    xck = get_xk(lc, w)
                    ps = pj_pool.tile([128, 512], f32, tag="pj")
                    for dt in range(ND):
                        nc.tensor.matmul(
                            ps[:, :w],
                            lhsT=wk_sb[:, dt * EH + eb * 128 : dt * EH + (eb + 1) * 128],
                            rhs=xck[:, dt * 512 : dt * 512 + w],
                            start=(dt == 0),
                            stop=(dt == ND - 1),
                        )
                    nc.vector.tensor_copy(
                        kt_sb[:, eb * LKS + lc : eb * LKS + lc + w], ps[:, :w]
                    )
                for lc in range(0, LQ, 512):
                    w = min(512, LQ - lc)
                    xcq = get_xq(lc, w)
                    ps = pj_pool.tile([128, 512], f32, tag="pj")
                    for dt in range(ND):
                        nc.tensor.matmul(
                            ps[:, :w],
                            lhsT=wq_sb[:, dt * EH + eb * 128 : dt * EH + (eb + 1) * 128],
                            rhs=xcq[:, dt * 512 : dt * 512 + w],
                            start=(dt == 0),
                            stop=(dt == ND - 1),
                        )
                    nc.vector.tensor_copy(
                        qt_sb[:, eb * LQS + lc : eb * LQS + lc + w], ps[:, :w]
                    )

            # ---- attention, with projection of the NEXT head pair
            # interleaved so it hides under this pair's ScalarE exps ----
            # lq handled in PAIRS of 128-blocks: scores at N=256 halve the
            # PE matmul/LDW count; T persists per pair-iteration and the
            # two AV passes share the 2 accumulator banks sequentially.
            proj_kq(0)
            proj_v()
            for hp in range(NEB):
                hA, hB = 2 * hp, 2 * hp + 1
                for lqs in range(0, LQ, 256):
                    w = min(256, LQ - lqs)
                    nlqb = w // 128
                    tA = t_pool.tile([128, NTK * 256], bf16, tag="t")
                    tB = t_pool.tile([128, NTK * 256], bf16, tag="t")
                    for (t0, tn) in quads:
                        psA = pp_pool.tile([128, 1024], f32, tag="sq")
                        psB = pp_pool.tile([128, 1024], f32, tag="sq")
                        for j in range(tn):
                            tt = t0 + j
                            nc.tensor.matmul(
                                psA[:, j * w : (j + 1) * w],
                                lhsT=kt_sb[0:64, hp * LKS + tt * 128 : hp * LKS + (tt + 1) * 128],
                                rhs=qt_sb[0:64, hp * LQS + lqs : hp * LQS + lqs + w],
                                start=True,
                                stop=True,
                            )
                            nc.tensor.matmul(
                                psB[:, j * w : (j + 1) * w],
                                lhsT=kt_sb[64:128, hp * LKS + tt * 128 : hp * LKS + (tt + 1) * 128],
                                rhs=qt_sb[64:128, hp * LQS + lqs : hp * LQS + lqs + w],
                                start=True,
                                stop=True,
                            )
                        w_all = tn * w
                        nc.scalar.activation(
                            tA[:, t0 * w : t0 * w + w_all], psA[:, :w_all],
                            mybir.ActivationFunctionType.Exp,
                        )
                        nc.scalar.activation(
                            tB[:, t0 * w : t0 * w + w_all], psB[:, :w_all],
                            mybir.ActivationFunctionType.Exp,
                        )
                    for lb in range(nlqb):
                        pavA = pav_pool.tile([128, VW], f32, tag="av")
                        pavB = pav_pool.tile([128, VW], f32, tag="av")
                        for tt in range(NTK):
                            nc.tensor.matmul(
                                pavA[:, 0:VW],
                                lhsT=tA[:, tt * w + lb * 128 : tt * w + lb * 128 + 128],
                                rhs=v4[:, tt, hA, :],
                                start=(tt == 0),
                                stop=(tt == NTK - 1),
                            )
                            nc.tensor.matmul(
                                pavB[:, 0:VW],
                                lhsT=tB[:, tt * w + lb * 128 : tt * w + lb * 128 + 128],
                                rhs=v4[:, tt, hB, :],
                                start=(tt == 0),
                                stop=(tt == NTK - 1),
                            )
                        oA = o_pool.tile([128, VW], f32, tag="o")
                        oB = o_pool.tile([128, VW], f32, tag="o")
                        nc.vector.tensor_copy(oA[:, :], pavA[:, :])
                        nc.vector.tensor_copy(oB[:, :], pavB[:, :])
                        ls = lqs + lb * 128
                        nc.sync.dma_start(
                            outp[ls : ls + 128, hA * VW : (hA + 1) * VW], oA[:, :]
                        )
                        nc.sync.dma_start(
                            outp[ls : ls + 128, hB * VW : (hB + 1) * VW], oB[:, :]
                        )
                if hp + 1 < NEB:
                    proj_kq(hp + 1)

    nc.compile()
    return nc


def _get_nc(cfg):
    key = tuple(sorted(cfg.items()))
    if key not in _nc_cache:
        _nc_cache[key] = _build(cfg)
    return _nc_cache[key]


# ---------------------------------------------------------------------------
# Fast device path: ship one packed fp16 buffer (rows trimmed to the actual
# Q_len/V_len), all_gather on device over NeuronLink, build each core's Bass
# inputs in jit1, run the Bass NEFF in jit2 with on-device donated zeros,
# divide-and-pack valid rows in jit3, fetch only ~sum(Q_len) fp16 rows.
# The axon tunnel moves ~35MB/s, so wire bytes dominate wall time; this path
# cuts them from ~182MB to ~38MB per call.
# ---------------------------------------------------------------------------
_fast_cache = {}
VW = DH + 1


def _build_fast(cfg, qn, vlen_eff):
    """Build the 3-jit pipeline for static per-batch lengths.

    qn: per-batch valid Q rows; vlen_eff: per-batch effective V rows (>0).
    Returns (runner, layout): runner(packed_f16 [R,1024]) -> [sum(qn),1024] f16.
    """
    import jax
    import jax.numpy as jnp
    from jax import lax
    from jax.sharding import Mesh, PartitionSpec, NamedSharding
    import warnings
    with warnings.catch_warnings():
        warnings.simplefilter("ignore")
        try:
            from jax.experimental.shard_map import shard_map
        except ImportError:
            from functools import partial
            from jax import shard_map as _sm
            shard_map = partial(_sm)
    import concourse.bass2jax as b2j
    import concourse.mybir as mybir

    nc = _get_nc(cfg)
    NH, LQ, LK = cfg["NH"], cfg["LQ"], cfg["LK"]
    NTK = LK // 128
    assert nc.dbg_addr is None
    b2j.install_neuronx_cc_hook()

    # packed row layout: [K rows | V rows | Q rows | WQ | WK | WV]
    kofs, acc = [0] * B, 0
    for b in range(B):
        kofs[b] = acc
        acc += vlen_eff[b]
    KT = acc
    vofs = [KT + o for o in kofs]
    acc = 2 * KT
    qofs = [0] * B
    for b in range(B):
        qofs[b] = acc
        acc += qn[b]
    W0 = acc
    total = acc + 3 * 1024
    R = (total + 7) // 8 * 8
    # dynamic_slice must never clamp: after any region start there are
    # >= 3072 rows (the W region), and LQ,LK <= 2048 < 3072.
    layout = {"kofs": kofs, "vofs": vofs, "qofs": qofs, "W0": W0,
              "total": total, "R": R}

    devices = jax.devices()[:8]
    mesh = Mesh(np.asarray(devices), ("core",))
    sh_core = NamedSharding(mesh, PartitionSpec("core"))

    kofs_t = jnp.asarray(kofs, jnp.int32)
    vofs_t = jnp.asarray(vofs, jnp.int32)
    qofs_t = jnp.asarray(qofs, jnp.int32)
    vlen_t = jnp.asarray(vlen_eff, jnp.int32)

    def _prep(shard):  # [R//8, 1024] f16 per core
        buf = lax.all_gather(shard, "core", tiled=True)  # [R, 1024]
        c = lax.axis_index("core")
        b = c // 2
        hg = c % 2
        vl = vlen_t[b]
        k = lax.dynamic_slice(buf, (kofs_t[b], 0), (LK, 1024))
        v = lax.dynamic_slice(buf, (vofs_t[b], 0), (LK, 1024))
        q = lax.dynamic_slice(buf, (qofs_t[b], 0), (LQ, 1024))
        kvalid = jnp.arange(LK, dtype=jnp.int32) < vl
        v = jnp.where(kvalid[:, None], v, jnp.float16(0))
        wq = lax.dynamic_slice(buf, (W0, hg * 512), (1024, 512))
        wk = lax.dynamic_slice(buf, (W0 + 1024, hg * 512), (1024, 512))
        wv = lax.dynamic_slice(buf, (W0 + 2048, hg * 512), (1024, 512))
        # km[p, t*NH + h] = kvalid[t*128 + p]
        km = jnp.broadcast_to(
            kvalid.reshape(NTK, 128).T[:, :, None], (128, NTK, NH)
        ).reshape(128, NTK * NH).astype(jnp.bfloat16)
        zo = jnp.zeros((LQ, NH * VW), jnp.float32)
        return q.T, k.T, v.T, wq, wk, wv, km, zo

    jit1 = jax.jit(shard_map(
        _prep, mesh=mesh, in_specs=PartitionSpec("core"),
        out_specs=(PartitionSpec("core"),) * 8, check_rep=False))

    partition_name = (nc.partition_id_tensor.name
                      if nc.partition_id_tensor else None)
    in_names, out_names, out_avals = [], [], []
    for alloc in nc.m.functions[0].allocations:
        if not isinstance(alloc, mybir.MemoryLocationSet):
            continue
        name = alloc.memorylocations[0].name
        if alloc.kind == "ExternalInput":
            if name != partition_name:
                in_names.append(name)
        elif alloc.kind == "ExternalOutput":
            out_names.append(name)
            out_avals.append(jax.core.ShapedArray(
                tuple(alloc.tensor_shape), mybir.dt.np(alloc.dtype)))
    assert in_names == ["xq", "xk", "xv", "wq", "wk", "wv", "km"], in_names
    assert out_names == ["outp"], out_names
    n_params = len(in_names)
    in_names_all = in_names + out_names + (
        [partition_name] if partition_name else [])

    def _body(*args):
        operands = list(args)
        if partition_name is not None:
            operands.append(b2j.partition_id_tensor())
        outs = b2j._bass_exec_p.bind(
            *operands, out_avals=tuple(out_avals),
            in_names=tuple(in_names_all), out_names=tuple(out_names),
            lowering_input_output_aliases=(),
            sim_require_finite=True, sim_require_nnan=True, nc=nc)
        return tuple(outs)

    jit2 = jax.jit(shard_map(
        _body, mesh=mesh, in_specs=(PartitionSpec("core"),) * (n_params + 1),
        out_specs=(PartitionSpec("core"),), check_rep=False),
        donate_argnums=(n_params,), keep_unused=True)

    # NOTE: cross-shard packing (slicing shards + concatenating across
    # devices) emits a GSPMD program this backend cannot load, and one
    # failed LoadExecutable poisons later loads — keep jit3 shard-local.
    def _post(outp):  # [8*LQ, NH*VW] f32 sharded on rows
        a = outp.reshape(8, LQ, NH, VW)
        o = (a[..., :DH] / a[..., DH:DH + 1]).astype(jnp.float16)
        return o.reshape(8, LQ, NH * DH)

    jit3 = jax.jit(_post)

    def runner(packed):  # np [R, 1024] f16
        dbuf = jax.device_put(packed, sh_core)
        dins = jit1(dbuf)
        outs = jit2(*dins)
        po = jit3(outs[0])
        return np.asarray(po)

    return runner, layout


def _get_fast(cfg, qn, vlen_eff):
    key = (tuple(sorted(cfg.items())), tuple(qn), tuple(vlen_eff))
    if key not in _fast_cache:
        _fast_cache[key] = _build_fast(cfg, qn, vlen_eff)
    return _fast_cache[key]


def _kernel_fast(Q_seq, K_seq, V_seq, q_len, v_len, WQ, WK, WV, cfg):
    import time as _time

    NH = cfg["NH"]
    qn = [int(min(q_len[b], L, cfg["LQ"])) for b in range(B)]
    vlen_eff = [int(min(v_len[b], L) if v_len[b] > 0 else L) for b in range(B)]
    runner, lay = _get_fast(cfg, qn, vlen_eff)

    f16 = np.float16
    packed = np.zeros((lay["R"], 1024), f16)
    for b in range(B):
        n = vlen_eff[b]
        packed[lay["kofs"][b]:lay["kofs"][b] + n] = K_seq[b][:n].astype(f16)
        packed[lay["vofs"][b]:lay["vofs"][b] + n] = V_seq[b][:n].astype(f16)
        if qn[b]:
            packed[lay["qofs"][b]:lay["qofs"][b] + qn[b]] = (
                Q_seq[b][:qn[b]].astype(f16))
    W0 = lay["W0"]
    packed[W0:W0 + 1024] = WQ.astype(f16)
    packed[W0 + 1024:W0 + 2048] = WK.astype(f16)
    packed[W0 + 2048:W0 + 3072] = WV.astype(f16)

    global LAST_SPMD_WALL_NS
    t0 = _time.time()
    po = runner(packed)  # [8, LQ, NH*DH] f16
    LAST_SPMD_WALL_NS = int((_time.time() - t0) * 1e9)

    out = np.zeros((B, L, H * DH), np.float32)
    for b in range(B):
        if qn[b]:
            out[b, :qn[b], :NH * DH] = po[2 * b, :qn[b]]
            out[b, :qn[b], NH * DH:] = po[2 * b + 1, :qn[b]]
    return out


def _prep_core_inputs(Xq, Xk, Xv, Wq, Wk, Wv, vlen, cfg):
    """Host-side slicing/transposition/masking for one core.

    Xq/Xk/Xv: [L, D] fp32 for this batch; W*: [D, EH] slices for this
    core's heads; vlen: effective V_len (0 means "no mask").
    """
    NH, LQ, LK = cfg["NH"], cfg["LQ"], cfg["LK"]
    f16 = np.float16
    bf16 = ml_dtypes.bfloat16

    NTK = LK // 128
    xq = np.zeros((D, LQ), f16)
    xq[:, : min(LQ, L)] = Xq[: min(LQ, L)].T.astype(f16)
    xk = np.zeros((D, LK), f16)
    xv = np.zeros((D, LK), f16)
    n = min(LK, L) if vlen == 0 else min(LK, vlen)
    xk[:, :n] = Xk[:n].T.astype(f16)
    xv[:, :n] = Xv[:n].T.astype(f16)
    kmask = (np.arange(LK) < n).astype(np.float32)
    # device layout [128, NTK*NH]: km[p, t*NH + h] = kmask[t*128 + p]
    kmv = np.repeat(
        kmask.reshape(NTK, 128).T[:, :, None], NH, axis=2
    ).reshape(128, NTK * NH)
    return {
        "xq": xq,
        "xk": xk,
        "xv": xv,
        "wq": np.ascontiguousarray(Wq, dtype=f16),
        "wk": np.ascontiguousarray(Wk, dtype=f16),
        "wv": np.ascontiguousarray(Wv, dtype=f16),
        "km": kmv.astype(bf16),
    }


def kernel(Q_seq, K_seq, V_seq, Q_len, V_len, WQ, WK, WV):
    from concourse.bass_utils import run_bass_kernel_spmd

    Q_seq = np.asarray(Q_seq, np.float32)
    K_seq = np.asarray(K_seq, np.float32)
    V_seq = np.asarray(V_seq, np.float32)
    WQ = np.asarray(WQ, np.float32)
    WK = np.asarray(WK, np.float32)
    WV = np.asarray(WV, np.float32)
    q_len = np.asarray(Q_len).reshape(-1).astype(np.int64)
    v_len = np.asarray(V_len).reshape(-1).astype(np.int64)

    # LQ covers the largest Q_len (batch 2: 1748); rows beyond each
    # batch's Q_len are dropped host-side anyway. LK must cover V_len.
    lq_need = int(min(L, max(1, q_len.max())))
    lk_need = int(min(L, max(v_len.max(), 1)))
    if (v_len == 0).any():
        lk_need = L
    cfg = {
        "NH": 8,
        "LQ": ((lq_need + 127) // 128) * 128,
        "LK": ((lk_need + 127) // 128) * 128,
    }
    NH, LQ, LK = cfg["NH"], cfg["LQ"], cfg["LK"]

    if os.environ.get("NN_ATT_NO_FAST") != "1":
        try:
            return _kernel_fast(Q_seq, K_seq, V_seq, q_len, v_len,
                                WQ, WK, WV, cfg)
        except Exception:
            import traceback
            traceback.print_exc()

    nc = _get_nc(cfg)

    in_maps = []
    core_meta = []
    for b in range(B):
        for hg in range(2):
            e0, e1 = hg * NH * DH, (hg + 1) * NH * DH
            m = _prep_core_inputs(
                Q_seq[b], K_seq[b], V_seq[b],
                WQ[:, e0:e1], WK[:, e0:e1], WV[:, e0:e1],
                int(v_len[b]), cfg,
            )
            in_maps.append(m)
            core_meta.append((b, hg))

    import time as _time

    trace = os.environ.get("NN_ATT_TRACE") == "1"
    t_spmd = _time.time()
    try:
        res = run_bass_kernel_spmd(
            nc, in_maps, core_ids=list(range(8)), trace=trace,
            **({"trace_cores": list(range(8))} if trace else {}),
        )
    except Exception:
        if not trace:
            raise
        res = run_bass_kernel_spmd(nc, in_maps, core_ids=list(range(8)))
    global LAST_EXEC_NS, LAST_RESULT, LAST_SPMD_WALL_NS
    LAST_SPMD_WALL_NS = int((_time.time() - t_spmd) * 1e9)
    LAST_RESULT = res
    if res.exec_time_ns:
        LAST_EXEC_NS = int(res.exec_time_ns)

    out = np.zeros((B, L, H * DH), np.float32)
    for c, (b, hg) in enumerate(core_meta):
        arr = res.results[c]["outp"]  # [LQ, NH*VW]
        nq = min(int(q_len[b]), LQ, L)
        if nq <= 0:
            continue
        a = arr[:nq].reshape(nq, NH, VW)
        num = a[:, :, :DH]
        den = a[:, :, DH:DH + 1]
        o = num / den
        out[b, :nq, hg * NH * DH : (hg + 1) * NH * DH] = o.reshape(nq, NH * DH)
    return out

